# revision 20
# baseline (speedup 1.0000x reference)
"""Trainium2 Bass kernel for a single nGPT-style attention head.

Computation (see reference): fused QKV projection, RoPE over the full head
dim, L2-normalize q/k scaled by sqk, causal SDPA with scale sqrt(d_model).

Sharding: data-parallel over batch — 8 batch elements, one per NeuronCore.

v10 design notes:
  - q/k projections keep the weight stationary ([d, t] psum out); the v
    projection makes the x-chunk stationary so v lands in [t, e] layout
    directly — no transposes, no DRAM roundtrip.
  - 1/||q||, 1/||k|| fold in phase A: ones-row matmul broadcasts into
    PSUM + one DVE multiply each. The broadcast matmuls and the final
    RoPE combines are emitted one block late so the PE's in-order queue
    runs block j+1's projection matmuls while block j's Ln/Exp chain is
    still on ACT (kills a ~3us PE stall per block).
  - Attention strips process in pairs sharing a [128, 2, 512] PSUM tile;
    non-diagonal pairs take a single [128, 1024] exp call. Diagonal
    strips stream only their valid column suffix; boundary tiles get a
    DVE tri-mask.
  - x and W loads are split across the three DMA-issue queues
    (SP/ACT/GPSIMD) so transfers run on different DMA engines.
  - Single pinned ACT table (natural_log_exp_and_others).
  - PSUM (8 banks): A: q(2) k(1) v(1) nq(1) nk(1) bc(2);
    C: sc(2x2) po(2) pd(1) bcd(1).
"""

import numpy as np
import ml_dtypes

import concourse.bass as bass
import concourse.tile as tile
from concourse import bacc, mybir
from concourse.bass import ts, ds
from concourse.bass_utils import run_bass_kernel_spmd

# Surface compile-hook exceptions (the PJRT bridge swallows tracebacks).
try:
    import traceback
    import libneuronxla as _lnx

    if not getattr(_lnx, "_err_wrapped", False):
        _orig_cc = _lnx.neuronx_cc

        def _cc_wrapper(*a, **kw):
            try:
                return _orig_cc(*a, **kw)
            except BaseException:
                traceback.print_exc()
                raise

        _lnx.neuronx_cc = _cc_wrapper
        _lnx._err_wrapped = True
except Exception:
    pass

AFT = mybir.ActivationFunctionType
ALU = mybir.AluOpType
F32 = mybir.dt.float32
BF16 = mybir.dt.bfloat16

B, T_FULL, C, D = 8, 2048, 1024, 128
ROPE_BASE = 10000.0
P = 128
TB = 512  # t-block (tq block width, PSUM-bank free dim)
NCO = C // P  # contraction chunks for the QKV projection
H = P // 2


def _pin_act_table():
    """Leave natural_log_exp_and_others as the only candidate activation
    table (it serves every func this kernel uses: copy/ln/exp), so the
    table-load pass emits exactly one ACT_TABLE_LOAD instead of 1.5us
    reloads on every ln<->exp alternation. Positions in the cached dict
    are untouched, so act_func_set_id stays a valid act_info.json index."""
    from concourse.hw_specs import get_activation_tables
    tabs = get_activation_tables("gen3")
    keep = "natural_log_exp_and_others"
    if keep in tabs:
        for name, funcs in tabs.items():
            if name != keep:
                funcs.clear()


def build_nc(T=T_FULL, num_devices=8):
    from contextlib import ExitStack
    _pin_act_table()
    NTB = T // TB
    NKT = T // P
    nc = bacc.Bacc("TRN2", target_bir_lowering=False, debug=False,
                   num_devices=num_devices)

    xT = nc.dram_tensor("xT", [C, T], BF16, kind="ExternalInput").ap()
    WT = nc.dram_tensor("WT", [C, 3 * D], BF16, kind="ExternalInput").ap()
    cosF = nc.dram_tensor("cosF", [P, T], BF16, kind="ExternalInput").ap()
    sinF = nc.dram_tensor("sinF", [P, T], BF16, kind="ExternalInput").ap()
    tri = nc.dram_tensor("tri", [P, P], BF16, kind="ExternalInput").ap()
    sqk = nc.dram_tensor("sqk", [D, 1], F32, kind="ExternalInput").ap()
    onb = nc.dram_tensor("onb", [P, 1], BF16, kind="ExternalInput").ap()
    onr = nc.dram_tensor("onr", [1, P], BF16, kind="ExternalInput").ap()
    outT = nc.dram_tensor("outT", [D, T], BF16, kind="ExternalOutput").ap()

    xT_t = xT.rearrange("(co p) t -> p co t", p=P)
    WT_t = WT.rearrange("(co p) d -> p co d", p=P)

    with tile.TileContext(nc) as tc:
        with ExitStack() as ctx:
            const = ctx.enter_context(tc.tile_pool(name="const", bufs=1))
            wpool = ctx.enter_context(tc.tile_pool(name="wpool", bufs=3))

            # W and x loads split across the three DMA-issue queues.
            wt = const.tile([P, NCO, 3 * D], BF16)
            _qs = [nc.sync, nc.scalar, nc.gpsimd]
            for co in range(NCO):
                _qs[co % 3].dma_start(wt[:, co, :], WT_t[:, co, :])
            sqk_sb = const.tile([D, 1], F32)
            nc.gpsimd.dma_start(sqk_sb, sqk)
            ones_k = const.tile([P, 1], BF16)
            nc.gpsimd.dma_start(ones_k, onb)
            ones_r = const.tile([1, P], BF16)
            nc.gpsimd.dma_start(ones_r, onr)
            # (sqk * C^(1/4))^2 = sqrt(C) * sqk^2 — full logit scale, on q.
            sqk232 = const.tile([D, 1], F32)
            nc.vector.tensor_scalar_mul(sqk232, sqk_sb, float(C ** 0.25))
            nc.vector.tensor_mul(sqk232, sqk232, sqk232)

            qk = const.tile([P, 2 * T], BF16)    # q̃^T | k̃^T (both scaled)
            vt = const.tile([P, NKT, P], BF16)   # v tiles [tk, e]

            xts = []
            for j in range(NTB):
                xt = const.tile([P, NCO, TB], BF16)
                tsl = ds(j * TB, TB)
                nc.sync.dma_start(xt[:, 0:3, :], xT_t[:, 0:3, tsl])
                nc.scalar.dma_start(xt[:, 3:6, :], xT_t[:, 3:6, tsl])
                nc.gpsimd.dma_start(xt[:, 6:8, :], xT_t[:, 6:8, tsl])
                xts.append(xt)

            # big rope/mask tables after x on their queues (needed later)
            tri_sb = const.tile([P, P], BF16)
            nc.gpsimd.dma_start(tri_sb, tri)
            cos_sb = const.tile([P, T], BF16)
            nc.scalar.dma_start(cos_sb, cosF)
            sin_sb = const.tile([P, T], BF16)
            nc.gpsimd.dma_start(sin_sb, sinF)

            # ---------- Phase A: QKV + norms + RoPE (per block) ----------
            with ExitStack() as actx:
                ps_q = actx.enter_context(
                    tc.tile_pool(name="ps_q", bufs=2, space="PSUM"))
                ps_k = actx.enter_context(
                    tc.tile_pool(name="ps_k", bufs=2, space="PSUM"))
                ps_v = actx.enter_context(
                    tc.tile_pool(name="ps_v", bufs=2, space="PSUM"))
                ps_n = actx.enter_context(
                    tc.tile_pool(name="ps_n", bufs=1, space="PSUM"))
                ps_bc = actx.enter_context(
                    tc.tile_pool(name="ps_bc", bufs=1, space="PSUM"))

                deferred = [None] * NTB

                def emit_deferred(jj):
                    """Block jj's broadcast matmuls + final RoPE combines,
                    emitted one block late so the PE's in-order queue runs
                    block jj+1's projection matmuls while block jj's
                    Ln/Exp chain is still on ACT."""
                    invq, invk, m12, m12k = deferred[jj]
                    tsl = ds(jj * TB, TB)
                    with nc.named_scope(f"fin{jj}"):
                        bcq = ps_bc.tile([P, TB], F32, tag="bc")
                        nc.tensor.matmul(bcq, ones_r, invq,
                                         start=True, stop=True)
                        nc.vector.scalar_tensor_tensor(
                            out=qk[:, tsl], in0=m12, scalar=sqk232,
                            in1=bcq, op0=ALU.mult, op1=ALU.mult)
                        bck = ps_bc.tile([P, TB], F32, tag="bc")
                        nc.tensor.matmul(bck, ones_r, invk,
                                         start=True, stop=True)
                        nc.vector.tensor_mul(qk[:, ds(T + jj * TB, TB)],
                                             m12k, bck)

                for j in range(NTB):
                    tsl = ds(j * TB, TB)
                    with nc.named_scope(f"qkv{j}"):
                        psq = ps_q.tile([P, TB], F32, tag="q")
                        psk = ps_k.tile([P, TB], F32, tag="k")
                        for g, pg in ((0, psq), (1, psk)):
                            for co in range(NCO):
                                nc.tensor.matmul(
                                    pg, wt[:, co, ts(g, D)],
                                    xts[j][:, co, :],
                                    start=(co == 0), stop=(co == NCO - 1))
                        # v in [t, e] layout directly: x-chunk stationary.
                        psv = ps_v.tile([P, 4, P], F32, tag="v")
                        for c in range(4):
                            for co in range(NCO):
                                nc.tensor.matmul(
                                    psv[:, c, :],
                                    xts[j][:, co, ts(c, P)],
                                    wt[:, co, ts(2, D)],
                                    start=(co == 0), stop=(co == NCO - 1))
                        nc.vector.tensor_copy(vt[:, ds(4 * j, 4), :], psv)
                        qkraw = wpool.tile([P, 2, TB], BF16, tag="qkraw")
                        nc.scalar.activation(qkraw[:, 0, :], psq, AFT.Copy)
                        nc.scalar.activation(qkraw[:, 1, :], psk, AFT.Copy)

                    with nc.named_scope(f"norm{j}"):
                        sq = wpool.tile([P, 2, TB], BF16, tag="sq")
                        nc.vector.tensor_mul(sq[:, 0, :], qkraw[:, 0, :],
                                             qkraw[:, 0, :])
                        nc.vector.tensor_mul(sq[:, 1, :], qkraw[:, 1, :],
                                             qkraw[:, 1, :])
                        nq = ps_n.tile([1, TB], F32, tag="n")
                        nc.tensor.matmul(nq, ones_k, sq[:, 0, :],
                                         start=True, stop=True)
                        lnq = wpool.tile([1, TB], F32, tag="lnq")
                        nc.scalar.activation(lnq, nq, AFT.Ln)
                        invq = wpool.tile([1, TB], BF16, tag="invq")
                        nc.scalar.activation(invq, lnq, AFT.Exp, scale=-0.5)
                        nk = ps_n.tile([1, TB], F32, tag="n")
                        nc.tensor.matmul(nk, ones_k, sq[:, 1, :],
                                         start=True, stop=True)
                        lnk = wpool.tile([1, TB], F32, tag="lnk")
                        nc.scalar.activation(lnk, nk, AFT.Ln)
                        invk = wpool.tile([1, TB], BF16, tag="invk")
                        nc.scalar.activation(invk, lnk, AFT.Exp, scale=-0.5)

                    with nc.named_scope(f"rope{j}"):
                        rot = wpool.tile([P, 2, TB], BF16, tag="rot")
                        nc.sync.dma_start(rot[0:H, :, :], qkraw[H:P, :, :])
                        nc.sync.dma_start(rot[H:P, :, :], qkraw[0:H, :, :])

                        m1 = wpool.tile([P, TB], BF16, tag="m1")
                        nc.vector.tensor_mul(m1, qkraw[:, 0, :],
                                             cos_sb[:, tsl])
                        m2 = wpool.tile([P, TB], BF16, tag="m2")
                        nc.vector.tensor_mul(m2, rot[:, 0, :],
                                             sin_sb[:, tsl])
                        m12 = wpool.tile([P, TB], BF16, tag="m12")
                        nc.vector.tensor_add(m12, m1, m2)
                        m1k = wpool.tile([P, TB], BF16, tag="m1k")
                        nc.vector.tensor_mul(m1k, qkraw[:, 1, :],
                                             cos_sb[:, tsl])
                        m2k = wpool.tile([P, TB], BF16, tag="m2k")
                        nc.vector.tensor_mul(m2k, rot[:, 1, :],
                                             sin_sb[:, tsl])
                        m12k = wpool.tile([P, TB], BF16, tag="m12k")
                        nc.vector.tensor_add(m12k, m1k, m2k)

                    deferred[j] = (invq, invk, m12, m12k)
                    if j > 0:
                        emit_deferred(j - 1)
                emit_deferred(NTB - 1)

            # ---------- Phase C: causal attention ----------
            with ExitStack() as cctx:
                expool = cctx.enter_context(
                    tc.tile_pool(name="expool", bufs=3))
                ps_sc = cctx.enter_context(
                    tc.tile_pool(name="ps_sc", bufs=2, space="PSUM"))
                ps_o = cctx.enter_context(
                    tc.tile_pool(name="ps_o", bufs=2, space="PSUM"))
                ps_d = cctx.enter_context(
                    tc.tile_pool(name="ps_d", bufs=1, space="PSUM"))
                ps_b2 = cctx.enter_context(
                    tc.tile_pool(name="ps_b2", bufs=1, space="PSUM"))

                for J in range(NTB):
                    with nc.named_scope(f"att{J}"):
                        q_blk = qk[:, ts(J, TB)]
                        po = ps_o.tile([P, TB], F32, tag="o")
                        pd = ps_d.tile([1, TB], F32, tag="d")
                        npair = 2 * (J + 1)
                        for g in range(npair):
                            i0, i1 = 2 * g, 2 * g + 1
                            dr0, dr1 = i0 - 4 * J, i1 - 4 * J
                            off0 = P * dr0 if dr0 >= 0 else 0
                            off1 = P * dr1 if dr1 >= 0 else 0
                            w0, w1 = TB - off0, TB - off1
                            sc = ps_sc.tile([P, 2, TB], F32, tag="sc")
                            nc.tensor.matmul(
                                sc[:, 0, ds(off0, w0)],
                                qk[:, ds(T + P * i0, P)],
                                q_blk[:, ds(off0, w0)],
                                start=True, stop=True)
                            nc.tensor.matmul(
                                sc[:, 1, ds(off1, w1)],
                                qk[:, ds(T + P * i1, P)],
                                q_blk[:, ds(off1, w1)],
                                start=True, stop=True)
                            ex = expool.tile([P, 2, TB], BF16, tag="ex")
                            if dr1 < 0:
                                nc.scalar.activation(ex, sc, AFT.Exp)
                            else:
                                nc.scalar.activation(
                                    ex[:, 0, ds(off0, w0)],
                                    sc[:, 0, ds(off0, w0)], AFT.Exp)
                                nc.scalar.activation(
                                    ex[:, 1, ds(off1, w1)],
                                    sc[:, 1, ds(off1, w1)], AFT.Exp)
                                if dr0 >= 0:
                                    nc.vector.tensor_mul(
                                        ex[:, 0, ds(off0, P)],
                                        ex[:, 0, ds(off0, P)], tri_sb)
                                nc.vector.tensor_mul(
                                    ex[:, 1, ds(off1, P)],
                                    ex[:, 1, ds(off1, P)], tri_sb)
                            for s, i, off, w in ((0, i0, off0, w0),
                                                 (1, i1, off1, w1)):
                                nc.tensor.matmul(
                                    po[:, ds(off, w)], vt[:, i, :],
                                    ex[:, s, ds(off, w)],
                                    start=(i == 0),
                                    stop=(i == 2 * npair - 1))
                                nc.tensor.matmul(
                                    pd[:, ds(off, w)], ones_k,
                                    ex[:, s, ds(off, w)],
                                    start=(i == 0),
                                    stop=(i == 2 * npair - 1))

                        lnd = wpool.tile([1, TB], F32, tag="lnd")
                        nc.scalar.activation(lnd, pd, AFT.Ln)
                        invd = wpool.tile([1, TB], BF16, tag="invd")
                        nc.scalar.activation(invd, lnd, AFT.Exp, scale=-1.0)
                        bcd = ps_b2.tile([P, TB], F32, tag="bcd")
                        nc.tensor.matmul(bcd, ones_r, invd,
                                         start=True, stop=True)
                        bcs = wpool.tile([P, TB], BF16, tag="bcs")
                        nc.vector.tensor_copy(bcs, bcd)
                        ob = wpool.tile([P, TB], BF16, tag="ob")
                        nc.vector.tensor_mul(ob, po, bcs)
                        nc.gpsimd.dma_start(outT[:, ts(J, TB)], ob)

    nc.compile()
    return nc


def _host_tables(T):
    d = D
    inv_freq = 1.0 / (ROPE_BASE ** (np.arange(0, d, 2, dtype=np.float64) / d))
    t = np.arange(T, dtype=np.float64)
    freqs = np.outer(inv_freq, t)  # [d/2, T]
    emb = np.concatenate([freqs, freqs], axis=0)  # [d, T]
    cos1 = np.cos(emb)
    sin1 = np.sin(emb)
    # sign of rotate_half folded into the table: rot is built with plain
    # copies, and sin rows 0:d/2 carry the minus sign instead.
    sin1[: d // 2, :] *= -1.0
    cosF = np.ascontiguousarray(cos1).astype(ml_dtypes.bfloat16)
    sinF = np.ascontiguousarray(sin1).astype(ml_dtypes.bfloat16)
    a = np.arange(P)
    tri = (a[None, :] >= a[:, None]).astype(ml_dtypes.bfloat16)  # [tk, tq]
    return cosF, sinF, tri


TRACE = False
LAST_EXEC_NS = None
LAST_TRACE = None
LAST_INSTS = None


def kernel(x, W_qkv, sqk):
    global LAST_EXEC_NS, LAST_TRACE, LAST_INSTS
    T = x.shape[1]
    cosF, sinF, tri = _host_tables(T)
    WT = np.ascontiguousarray(np.asarray(W_qkv).T).astype(ml_dtypes.bfloat16)
    sqk2 = np.ascontiguousarray(
        np.asarray(sqk).reshape(D, 1)).astype(np.float32)
    in_maps = []
    for b in range(B):
        in_maps.append({
            "xT": np.ascontiguousarray(
                np.asarray(x[b]).T).astype(ml_dtypes.bfloat16),
            "WT": WT,
            "cosF": cosF,
            "sinF": sinF,
            "tri": tri,
            "sqk": sqk2,
            "onb": np.ones((P, 1), ml_dtypes.bfloat16),
            "onr": np.ones((1, P), ml_dtypes.bfloat16),
        })
    nc = build_nc(T=T, num_devices=B)
    res = run_bass_kernel_spmd(nc, in_maps, core_ids=list(range(B)),
                               trace=TRACE)
    LAST_EXEC_NS = res.exec_time_ns
    LAST_TRACE = (res.instructions_and_trace[1]
                  if res.instructions_and_trace else None)
    LAST_INSTS = (res.instructions_and_trace[0]
                  if res.instructions_and_trace else None)
    out = np.stack([r["outT"].T for r in res.results])  # [B, T, D]
    return np.ascontiguousarray(out).astype(np.float32)
